# revision 6
# baseline (speedup 1.0000x reference)
"""Trainium2 Bass kernel for nn_DihedralAngleLayer (planar v3).

Input:  x [2_000_000, 42] f32 (14 atoms x 3 coords per row),
        mask_matrix [4, 14] f32 one-hot carbon selector.
Output: dihedral angle per row, [2_000_000] f32.

Data-parallel across 8 NeuronCores; rows padded to 8*128*1960 and split.
Per core, rows are partition-major: partition p owns rows [p*Q, (p+1)*Q).

Per uniform block (G=196 rows/partition) the row-major tile is planarized by
TWO transposing copies on the Scalar/ACT engine (carbon pairs (0,4) and
(7,11) both have uniform column stride, so each copy moves 2 carbons x 3
components in one 3-dim-AP op), then ONE contiguous DVE subtract forms all
nine bond-vector planes at once: (u,v,w) = carbons[1:4] - carbons[0:3].
This keeps the expensive strided access patterns off the Vector engine
(f32 TT has no DVE fast mode; strided single-element runs cost ~2x).

Phase 2 per chunk of blocks (planar, unit-stride, 18 DVE instrs):
    NB = v x w,  NA = u x v          (pairwise plane mults, P1-P2)
    q3 = v*v (ACT Square),  d3 = u*NB,  x3 = NA*NB
    (q, det, xx) = joint 3-group pair-sum of [q3|d3|x3]   (2 strided adds)
    yy = det * sqrt(q)  (sqrt on ACT)
    tail (direct atan2, ACT arctan saturates correctly for huge args):
        t = yy * recip(xx);  A = arctan(t)
        phi = A + (pi/2)*(sgy - sgx*sgy)
GPSIMD stays fully idle: concurrent GPSIMD tensor work inflates DVE ~2.4x
(SBUF port contention, measured).  Input DMAs on nc.sync (HWDGE), output
stores on nc.scalar.  Chunks [2,2,2,2,1,1] pipeline phase 2 against the
DMA stream with small tail chunks to bound post-last-DMA exposure.
"""

import numpy as np

import concourse.bacc as bacc
import concourse.bass as bass
import concourse.mybir as mybir
from concourse.bass_utils import run_bass_kernel_spmd
from concourse.tile import TileContext

AF = mybir.ActivationFunctionType
OP = mybir.AluOpType
F32 = mybir.dt.float32

PI = float(np.pi)

N_CORES = 8
G = 196
T = 10
Q = G * T                   # 1960 rows per partition
ROWS_PER_CORE = 128 * Q     # 250880
CHUNKS = [2, 2, 2, 2, 1, 1]

BS = 9 * G                  # b-plane block stride
SP = 20                     # scratch planes per block
SS = SP * G                 # scratch block stride


def _ap(base, off, dims):
    return bass.AP(
        base.tensor, base.offset + off, [list(base.ap[0])] + [list(d) for d in dims]
    )


def _emit_ph1(nc, bba, cpa, xta, b, c0, c1, c2, c3):
    """ACT transposing copies -> 12 carbon planes; DVE mega-sub -> 9 b-planes."""
    s, v = nc.scalar, nc.vector
    # carbons (0,1) at cols c0,c1 and (2,3) at c2,c3: uniform stride per pair
    s.activation(
        _ap(cpa, 0, [[3 * G, 2], [G, 3], [1, G]]),
        _ap(xta, c0, [[c1 - c0, 2], [1, 3], [42, G]]),
        AF.Copy,
    )
    s.activation(
        _ap(cpa, 6 * G, [[3 * G, 2], [G, 3], [1, G]]),
        _ap(xta, c2, [[c3 - c2, 2], [1, 3], [42, G]]),
        AF.Copy,
    )
    # (u,v,w) = carbons[1:4] - carbons[0:3], all contiguous
    v.tensor_tensor(
        _ap(bba, b * BS, [[1, 9 * G]]),
        _ap(cpa, 3 * G, [[1, 9 * G]]),
        _ap(cpa, 0, [[1, 9 * G]]),
        OP.subtract,
    )


def _emit_ph2(nc, bba, sca, oba, y, b0, nb):
    """Cross-form dihedral + direct atan2 for chunk [b0, b0+nb) blocks."""
    v, s = nc.vector, nc.scalar

    def bap(plane, n=1, ps=1):
        dims = [[BS, nb]] + ([[ps * G, n]] if n > 1 else []) + [[1, G]]
        return _ap(bba, b0 * BS + plane * G, dims)

    def sap(plane, n=1, ps=1):
        dims = [[SS, nb]] + ([[ps * G, n]] if n > 1 else []) + [[1, G]]
        return _ap(sca, plane * G, dims)

    # q3 = v*v -> X(0-2)
    s.activation(sap(0, 3), bap(3, 3), AF.Square)
    # NB = v x w -> C(9-11):  P1 -> C, P2 -> B(12-14), then C = C - B
    v.tensor_tensor(sap(9, 2), bap(4, 2), bap(8, 2, -2), OP.mult)   # vy*wz, vz*wx
    v.tensor_tensor(sap(12, 2), bap(5, 2, -2), bap(7, 2), OP.mult)  # vz*wy, vx*wz
    v.tensor_tensor(sap(11, 2, 3), bap(3, 2), bap(7, 2, -1), OP.mult)  # vx*wy, vy*wx
    v.tensor_tensor(sap(9, 3), sap(9, 3), sap(12, 3), OP.subtract)
    # d3 = u * NB -> X(3-5)
    v.tensor_tensor(sap(3, 3), bap(0, 3), sap(9, 3), OP.mult)
    # NA = u x v -> B(12-14): P1 -> B, P2 -> (15-17), then B = B - P2
    v.tensor_tensor(sap(12, 2), bap(1, 2), bap(5, 2, -2), OP.mult)  # uy*vz, uz*vx
    v.tensor_tensor(sap(15, 2), bap(2, 2, -2), bap(4, 2), OP.mult)  # uz*vy, ux*vz
    v.tensor_tensor(sap(14, 2, 3), bap(0, 2), bap(4, 2, -1), OP.mult)  # ux*vy, uy*vx
    v.tensor_tensor(sap(12, 3), sap(12, 3), sap(15, 3), OP.subtract)
    # x3 = NA * NB -> X(6-8)
    v.tensor_tensor(sap(6, 3), sap(12, 3), sap(9, 3), OP.mult)
    # joint sums: (q, det, xx) -> (15, 16, 17)
    v.tensor_tensor(sap(15, 3), sap(0, 3, 3), sap(1, 3, 3), OP.add)
    v.tensor_tensor(sap(15, 3), sap(15, 3), sap(2, 3, 3), OP.add)
    # ACT burst (one sqrt-set visit): sq = sqrt(q); sgx; sgy = sign(det)
    # (sign(yy) == sign(det) since sqrt(q) > 0)
    s.activation(sap(18), sap(15), AF.Sqrt)
    s.activation(sap(0), sap(17), AF.Sign)             # sgx
    s.activation(sap(1), sap(16), AF.Sign)             # sgy
    # tail: phi = arctan(sqrt(q)*det/xx) + (pi/2)*(sgy - sgx*sgy)
    v.reciprocal_approx_fast(sap(2), sap(17))          # rd = 1/xx (no ACT dep)
    v.tensor_tensor(sap(3), sap(16), sap(2), OP.mult)  # t1 = det * rd
    v.tensor_tensor(sap(3), sap(3), sap(18), OP.mult)  # t = t1 * sq
    s.activation(sap(4), sap(3), AF.Arctan)            # A
    v.tensor_tensor(sap(5), sap(0), sap(1), OP.mult)   # s2 = sgx*sgy
    v.tensor_tensor(sap(5), sap(1), sap(5), OP.subtract)  # t2 = sgy - s2
    v.scalar_tensor_tensor(
        _ap(oba, 0, [[G, nb], [1, G]]), sap(5), PI / 2, sap(4), OP.mult, OP.add
    )
    nc.scalar.dma_start(
        out=y.rearrange("(p q) -> p q", p=128)[:, b0 * G : (b0 + nb) * G],
        in_=_ap(oba, 0, [[1, nb * G]]),
    )


def build_kernel(atoms):
    c0, c1, c2, c3 = (3 * int(a) for a in atoms)
    nc = bacc.Bacc("TRN2", target_bir_lowering=False, debug=False)
    x = nc.dram_tensor("x", [ROWS_PER_CORE, 42], F32, kind="ExternalInput")
    y = nc.dram_tensor("y", [ROWS_PER_CORE], F32, kind="ExternalOutput")
    xr = x.rearrange("(p q) c -> p q c", p=128)
    with TileContext(nc) as tc:
        with (
            tc.tile_pool(name="xp", bufs=2) as xp,
            tc.tile_pool(name="cp", bufs=2) as cp,
            tc.tile_pool(name="bp", bufs=1) as bp,
            tc.tile_pool(name="sp", bufs=1) as sp,
            tc.tile_pool(name="op", bufs=2) as op,
        ):
            bb = bp.tile([128, 9 * G * T], F32, tag="b")
            bba = bb[:]
            b = 0
            for nb in CHUNKS:
                for _ in range(nb):
                    xt = xp.tile([128, G * 42], F32, tag="x")
                    nc.gpsimd.dma_start(out=xt[:], in_=xr[:, b * G : (b + 1) * G, :])
                    ct = cp.tile([128, 12 * G], F32, tag="c")
                    _emit_ph1(nc, bba, ct[:], xt[:], b, c0, c1, c2, c3)
                    b += 1
                sc = sp.tile([128, SS * 2], F32, tag="sc")
                ob = op.tile([128, G * 2], F32, tag="o")
                _emit_ph2(nc, bba, sc[:], ob[:], y, b - nb, nb)
    nc.finalize()
    return nc


_CACHE = {}


def _get_nc(atoms):
    key = tuple(int(a) for a in atoms)
    if key not in _CACHE:
        _CACHE[key] = build_kernel(key)
    return _CACHE[key]


def run(x, atoms=(0, 4, 7, 11), **spmd_kwargs):
    """x: [B, 42] f32. Returns (y [B] f32, BassKernelResults)."""
    x = np.ascontiguousarray(np.asarray(x, dtype=np.float32))
    B = x.shape[0]
    total = N_CORES * ROWS_PER_CORE
    if B < total:
        # pad with replicated leading rows (valid, non-degenerate data)
        reps = -(-(total - B) // B)
        x = np.concatenate([x] + [x] * reps, axis=0)[:total]
    nc = _get_nc(atoms)
    shards = x.reshape(N_CORES, ROWS_PER_CORE, 42)
    in_maps = [{"x": shards[i]} for i in range(N_CORES)]
    res = run_bass_kernel_spmd(nc, in_maps, core_ids=list(range(N_CORES)), **spmd_kwargs)
    y = np.concatenate([r["y"] for r in res.results])[:B]
    return np.asarray(y, dtype=np.float32), res


def kernel(x, mask_matrix):
    mask = np.asarray(mask_matrix)
    atoms = tuple(int(i) for i in np.argmax(mask, axis=1))
    y, _ = run(x, atoms=atoms)
    return y


# revision 7
# speedup vs baseline: 1.0408x; 1.0408x over previous
"""Trainium2 Bass kernel for nn_DihedralAngleLayer (planar v5).

Input:  x [2_000_000, 42] f32 (14 atoms x 3 coords per row),
        mask_matrix [4, 14] f32 one-hot carbon selector.
Output: dihedral angle per row, [2_000_000] f32.

Data-parallel across 8 NeuronCores; rows padded to 8*128*1960 and split.
Per core, rows are partition-major: partition p owns rows [p*Q, (p+1)*Q).

Per uniform block (G=196 rows/partition) the row-major tile is planarized by
TWO transposing copies on the Scalar/ACT engine (carbon pairs share a
uniform column stride, so each copy moves 2 carbons x 3 components in one
3-dim-AP op), then ONE contiguous DVE subtract forms all nine bond-vector
planes: (u,v,w) = carbons[1:4] - carbons[0:3].  This keeps strided access
patterns off the Vector engine (f32 TT has no DVE fast mode; strided
single-element runs cost ~2x, measured).

Phase 2 per chunk of blocks (planar unit-stride, 16 DVE instrs):
    NB = v x w,  NA = u x v            (pairwise plane mults, P1-P2)
    q3 = v*v (ACT Square), d3 = u*NB, x3 = NA*NB (in-place over NB)
    (q, det, xx) = joint 3-group pair-sum                (2 strided adds)
    one ACT burst: sq = sqrt(q), sgx = sign(xx), sgy = sign(det)
    tail (direct atan2 -- the ACT arctan table saturates correctly for
    huge arguments):  t = det*recip(xx)*sq;  A = arctan(t)
        phi = A + (pi/2)*(sgy - sgx*sgy)

The emission is software-pipelined: chunk c's post-arctan tail is emitted
after chunk c+1's phase-1 and head, so the DVE never stalls on the ACT
round-trips (sqrt/arctan sit behind ~1.3us table swaps: no ACT table set
holds both sqrt and arctan).  Input loads alternate between the two HWDGE
rings (nc.sync / nc.scalar) so two DMAs stream concurrently; GPSIMD stays
fully idle (concurrent GPSIMD tensor/SWDGE work measurably degrades DVE
and the DMA stream).
"""

import numpy as np

import concourse.bacc as bacc
import concourse.bass as bass
import concourse.mybir as mybir
from concourse.bass_utils import run_bass_kernel_spmd
from concourse.tile import TileContext

AF = mybir.ActivationFunctionType
OP = mybir.AluOpType
F32 = mybir.dt.float32

PI = float(np.pi)

N_CORES = 8
G = 196
T = 10
Q = G * T                   # 1960 rows per partition
ROWS_PER_CORE = 128 * Q     # 250880
CHUNKS = [2, 2, 2, 2, 1, 1]

BS = 9 * G                  # b-plane block stride within a chunk tile
SP = 15                     # head scratch planes per block
SS = SP * G                 # head scratch block stride
TP = 10                     # tail planes per chunk


def _ap(base, off, dims):
    return bass.AP(
        base.tensor, base.offset + off, [list(base.ap[0])] + [list(d) for d in dims]
    )


def _emit_ph1(nc, bba, cpa, xta, j, c0, c1, c2, c3):
    """ACT transposing copies -> 12 carbon planes; DVE mega-sub -> 9 b-planes
    at local block index j of the chunk's b-tile."""
    s, v = nc.scalar, nc.vector
    s.activation(
        _ap(cpa, 0, [[3 * G, 2], [G, 3], [1, G]]),
        _ap(xta, c0, [[c1 - c0, 2], [1, 3], [42, G]]),
        AF.Copy,
    )
    s.activation(
        _ap(cpa, 6 * G, [[3 * G, 2], [G, 3], [1, G]]),
        _ap(xta, c2, [[c3 - c2, 2], [1, 3], [42, G]]),
        AF.Copy,
    )
    v.tensor_tensor(
        _ap(bba, j * BS, [[1, 9 * G]]),
        _ap(cpa, 3 * G, [[1, 9 * G]]),
        _ap(cpa, 0, [[1, 9 * G]]),
        OP.subtract,
    )


class Chunk:
    def __init__(self, b0, nb, bba, sca, tla, oba):
        self.b0, self.nb = b0, nb
        self.bba, self.sca, self.tla, self.oba = bba, sca, tla, oba


def _bap(ch, plane, n=1, ps=1):
    dims = [[BS, ch.nb]] + ([[ps * G, n]] if n > 1 else []) + [[1, G]]
    return _ap(ch.bba, plane * G, dims)


def _sap(ch, plane, n=1, ps=1):
    dims = [[SS, ch.nb]] + ([[ps * G, n]] if n > 1 else []) + [[1, G]]
    return _ap(ch.sca, plane * G, dims)


def _tap(ch, plane):
    return _ap(ch.tla, plane * ch.nb * G, [[G, ch.nb], [1, G]])


def _emit_head(nc, ch):
    """Head: everything up to (and including) recip(xx).
    Scratch: q3=A(0-2), d3=D(3-5), NB=C(6-8), NA=B(9-11), P2=P(12-14).
    Tail tile: q(0) det(1) xx(2) sq(3) sgx(4) sgy(5) rd(6) t(7) A(8) s2(9)."""
    v, s = nc.vector, nc.scalar
    s.activation(_sap(ch, 0, 3), _bap(ch, 3, 3), AF.Square)                 # q3
    v.tensor_tensor(_sap(ch, 6, 2), _bap(ch, 4, 2), _bap(ch, 8, 2, -2), OP.mult)
    v.tensor_tensor(_sap(ch, 12, 2), _bap(ch, 5, 2, -2), _bap(ch, 7, 2), OP.mult)
    v.tensor_tensor(_sap(ch, 8, 2, 6), _bap(ch, 3, 2), _bap(ch, 7, 2, -1), OP.mult)
    v.tensor_tensor(_sap(ch, 6, 3), _sap(ch, 6, 3), _sap(ch, 12, 3), OP.subtract)  # NB
    v.tensor_tensor(_sap(ch, 3, 3), _bap(ch, 0, 3), _sap(ch, 6, 3), OP.mult)       # d3
    v.tensor_tensor(_sap(ch, 9, 2), _bap(ch, 1, 2), _bap(ch, 5, 2, -2), OP.mult)
    v.tensor_tensor(_sap(ch, 12, 2), _bap(ch, 2, 2, -2), _bap(ch, 4, 2), OP.mult)
    v.tensor_tensor(_sap(ch, 11, 2, 3), _bap(ch, 0, 2), _bap(ch, 4, 2, -1), OP.mult)
    v.tensor_tensor(_sap(ch, 9, 3), _sap(ch, 9, 3), _sap(ch, 12, 3), OP.subtract)  # NA
    v.tensor_tensor(_sap(ch, 6, 3), _sap(ch, 9, 3), _sap(ch, 6, 3), OP.mult)       # x3
    # joint sums (plane, block, row) on both sides
    sum_dst = _ap(ch.tla, 0, [[ch.nb * G, 3], [G, ch.nb], [1, G]])
    v.tensor_tensor(
        sum_dst,
        _ap(ch.sca, 0, [[3 * G, 3], [SS, ch.nb], [1, G]]),
        _ap(ch.sca, G, [[3 * G, 3], [SS, ch.nb], [1, G]]),
        OP.add,
    )
    v.tensor_tensor(
        sum_dst, sum_dst, _ap(ch.sca, 2 * G, [[3 * G, 3], [SS, ch.nb], [1, G]]), OP.add
    )
    # single sqrt-set ACT visit; sign(yy) == sign(det)
    s.activation(_tap(ch, 3), _tap(ch, 0), AF.Sqrt)
    s.activation(_tap(ch, 4), _tap(ch, 2), AF.Sign)
    s.activation(_tap(ch, 5), _tap(ch, 1), AF.Sign)
    v.reciprocal_approx_fast(_tap(ch, 6), _tap(ch, 2))


def _emit_tail_a(nc, ch):
    """t = det*rd*sq; s2/t2; arctan (ACT ring, hidden behind next head)."""
    v, s = nc.vector, nc.scalar
    v.tensor_tensor(_tap(ch, 7), _tap(ch, 1), _tap(ch, 6), OP.mult)
    v.tensor_tensor(_tap(ch, 7), _tap(ch, 7), _tap(ch, 3), OP.mult)
    v.tensor_tensor(_tap(ch, 9), _tap(ch, 4), _tap(ch, 5), OP.mult)
    v.tensor_tensor(_tap(ch, 9), _tap(ch, 5), _tap(ch, 9), OP.subtract)
    s.activation(_tap(ch, 8), _tap(ch, 7), AF.Arctan)


def _emit_tail_b(nc, ch, y):
    v = nc.vector
    v.scalar_tensor_tensor(
        _ap(ch.oba, 0, [[G, ch.nb], [1, G]]),
        _tap(ch, 9), PI / 2, _tap(ch, 8), OP.mult, OP.add,
    )
    nc.scalar.dma_start(
        out=y.rearrange("(p q) -> p q", p=128)[:, ch.b0 * G : (ch.b0 + ch.nb) * G],
        in_=_ap(ch.oba, 0, [[1, ch.nb * G]]),
    )


def build_kernel(atoms):
    c0, c1, c2, c3 = (3 * int(a) for a in atoms)
    nc = bacc.Bacc("TRN2", target_bir_lowering=False, debug=False)
    x = nc.dram_tensor("x", [ROWS_PER_CORE, 42], F32, kind="ExternalInput")
    y = nc.dram_tensor("y", [ROWS_PER_CORE], F32, kind="ExternalOutput")
    xr = x.rearrange("(p q) c -> p q c", p=128)
    with TileContext(nc) as tc:
        with (
            tc.tile_pool(name="xp", bufs=2) as xp,
            tc.tile_pool(name="cp", bufs=2) as cp,
            tc.tile_pool(name="bp", bufs=3) as bp,
            tc.tile_pool(name="sp", bufs=1) as sp,
            tc.tile_pool(name="tp", bufs=2) as tp,
            tc.tile_pool(name="op", bufs=2) as op,
        ):
            prev = None
            b = 0
            for nb in CHUNKS:
                bb = bp.tile([128, BS * 2], F32, tag="b")
                for j in range(nb):
                    xt = xp.tile([128, G * 42], F32, tag="x")
                    ring = nc.sync if (b % 2 == 0) else nc.scalar
                    ring.dma_start(out=xt[:], in_=xr[:, b * G : (b + 1) * G, :])
                    ct = cp.tile([128, 12 * G], F32, tag="c")
                    _emit_ph1(nc, bb[:], ct[:], xt[:], j, c0, c1, c2, c3)
                    b += 1
                sc = sp.tile([128, SS * 2], F32, tag="sc")
                tl = tp.tile([128, TP * G * 2], F32, tag="tl")
                ob = op.tile([128, G * 2], F32, tag="o")
                ch = Chunk(b - nb, nb, bb[:], sc[:], tl[:], ob[:])
                if prev is not None:
                    _emit_tail_a(nc, prev)
                _emit_head(nc, ch)
                if prev is not None:
                    _emit_tail_b(nc, prev, y)
                prev = ch
            _emit_tail_a(nc, prev)
            _emit_tail_b(nc, prev, y)
    nc.finalize()
    return nc


_CACHE = {}


def _get_nc(atoms):
    key = tuple(int(a) for a in atoms)
    if key not in _CACHE:
        _CACHE[key] = build_kernel(key)
    return _CACHE[key]


def run(x, atoms=(0, 4, 7, 11), **spmd_kwargs):
    """x: [B, 42] f32. Returns (y [B] f32, BassKernelResults)."""
    x = np.ascontiguousarray(np.asarray(x, dtype=np.float32))
    B = x.shape[0]
    total = N_CORES * ROWS_PER_CORE
    if B < total:
        # pad with replicated leading rows (valid, non-degenerate data)
        reps = -(-(total - B) // B)
        x = np.concatenate([x] + [x] * reps, axis=0)[:total]
    nc = _get_nc(atoms)
    shards = x.reshape(N_CORES, ROWS_PER_CORE, 42)
    in_maps = [{"x": shards[i]} for i in range(N_CORES)]
    res = run_bass_kernel_spmd(nc, in_maps, core_ids=list(range(N_CORES)), **spmd_kwargs)
    y = np.concatenate([r["y"] for r in res.results])[:B]
    return np.asarray(y, dtype=np.float32), res


def kernel(x, mask_matrix):
    mask = np.asarray(mask_matrix)
    atoms = tuple(int(i) for i in np.argmax(mask, axis=1))
    y, _ = run(x, atoms=atoms)
    return y


# revision 12
# speedup vs baseline: 1.0833x; 1.0409x over previous
"""Trainium2 Bass kernel for nn_DihedralAngleLayer (planar v5).

Input:  x [2_000_000, 42] f32 (14 atoms x 3 coords per row),
        mask_matrix [4, 14] f32 one-hot carbon selector.
Output: dihedral angle per row, [2_000_000] f32.

Data-parallel across 8 NeuronCores; rows padded to 8*128*1960 and split.
Per core, rows are partition-major: partition p owns rows [p*Q, (p+1)*Q).

Per uniform block (G=196 rows/partition) the row-major tile is planarized by
TWO transposing copies on the Scalar/ACT engine (carbon pairs share a
uniform column stride, so each copy moves 2 carbons x 3 components in one
3-dim-AP op), then ONE contiguous DVE subtract forms all nine bond-vector
planes: (u,v,w) = carbons[1:4] - carbons[0:3].  This keeps strided access
patterns off the Vector engine (f32 TT has no DVE fast mode; strided
single-element runs cost ~2x, measured).

Phase 2 per chunk of blocks (planar unit-stride, 16 DVE instrs):
    NB = v x w,  NA = u x v            (pairwise plane mults, P1-P2)
    q3 = v*v (ACT Square), d3 = u*NB, x3 = NA*NB (in-place over NB)
    (q, det, xx) = joint 3-group pair-sum                (2 strided adds)
    one ACT burst: sq = sqrt(q), sgx = sign(xx), sgy = sign(det)
    tail (direct atan2 -- the ACT arctan table saturates correctly for
    huge arguments):  t = det*recip(xx)*sq;  A = arctan(t)
        phi = A + (pi/2)*(sgy - sgx*sgy)

The emission is software-pipelined: chunk c's post-arctan tail is emitted
after chunk c+1's phase-1 and head, so the DVE never stalls on the ACT
round-trips (sqrt/arctan sit behind ~1.3us table swaps: no ACT table set
holds both sqrt and arctan).  Input loads alternate between the two HWDGE
rings (nc.sync / nc.scalar) so two DMAs stream concurrently; GPSIMD stays
fully idle (concurrent GPSIMD tensor/SWDGE work measurably degrades DVE
and the DMA stream).
"""

import numpy as np

import concourse.bacc as bacc
import concourse.bass as bass
import concourse.mybir as mybir
from concourse.bass_utils import run_bass_kernel_spmd
from concourse.tile import TileContext

AF = mybir.ActivationFunctionType
OP = mybir.AluOpType
F32 = mybir.dt.float32

PI = float(np.pi)

N_CORES = 8
G = 196
T = 10
Q = G * T                   # 1960 rows per partition
ROWS_PER_CORE = 128 * Q     # 250880
CHUNKS = [1, 2, 2, 2, 2, 1]

BS = 9 * G                  # b-plane block stride within a chunk tile
SP = 15                     # head scratch planes per block
SS = SP * G                 # head scratch block stride
TP = 10                     # tail planes per chunk


def _ap(base, off, dims):
    return bass.AP(
        base.tensor, base.offset + off, [list(base.ap[0])] + [list(d) for d in dims]
    )


def _emit_ph1(nc, bba, cpa, xta, j, c0, c1, c2, c3, halves=False):
    """ACT transposing copies -> 12 carbon planes; DVE mega-sub -> 9 b-planes
    at local block index j of the chunk's b-tile.  With halves=True the
    copies are row-split so they start at half-block DMA arrival."""
    s, v = nc.scalar, nc.vector
    parts = ((0, G // 2), (G // 2, G - G // 2)) if halves else ((0, G),)
    for r0, rn in parts:
        s.activation(
            _ap(cpa, r0, [[3 * G, 2], [G, 3], [1, rn]]),
            _ap(xta, c0 + r0 * 42, [[c1 - c0, 2], [1, 3], [42, rn]]),
            AF.Copy,
        )
        s.activation(
            _ap(cpa, 6 * G + r0, [[3 * G, 2], [G, 3], [1, rn]]),
            _ap(xta, c2 + r0 * 42, [[c3 - c2, 2], [1, 3], [42, rn]]),
            AF.Copy,
        )
    v.tensor_tensor(
        _ap(bba, j * BS, [[1, 9 * G]]),
        _ap(cpa, 3 * G, [[1, 9 * G]]),
        _ap(cpa, 0, [[1, 9 * G]]),
        OP.subtract,
    )


class Chunk:
    def __init__(self, b0, nb, bba, sca, tla, oba):
        self.b0, self.nb = b0, nb
        self.bba, self.sca, self.tla, self.oba = bba, sca, tla, oba


def _bap(ch, plane, n=1, ps=1):
    dims = [[BS, ch.nb]] + ([[ps * G, n]] if n > 1 else []) + [[1, G]]
    return _ap(ch.bba, plane * G, dims)


def _sap(ch, plane, n=1, ps=1):
    dims = [[SS, ch.nb]] + ([[ps * G, n]] if n > 1 else []) + [[1, G]]
    return _ap(ch.sca, plane * G, dims)


def _tap(ch, plane):
    return _ap(ch.tla, plane * ch.nb * G, [[G, ch.nb], [1, G]])


def _emit_head(nc, ch):
    """Head: everything up to (and including) recip(xx).
    Scratch: q3=A(0-2), d3=D(3-5), NB=C(6-8), NA=B(9-11), P2=P(12-14).
    Tail tile: q(0) det(1) xx(2) sq(3) sgx(4) sgy(5) rd(6) t(7) A(8) s2(9)."""
    v, s = nc.vector, nc.scalar
    s.activation(_sap(ch, 0, 3), _bap(ch, 3, 3), AF.Square)                 # q3
    v.tensor_tensor(_sap(ch, 6, 2), _bap(ch, 4, 2), _bap(ch, 8, 2, -2), OP.mult)
    v.tensor_tensor(_sap(ch, 12, 2), _bap(ch, 5, 2, -2), _bap(ch, 7, 2), OP.mult)
    v.tensor_tensor(_sap(ch, 8, 2, 6), _bap(ch, 3, 2), _bap(ch, 7, 2, -1), OP.mult)
    v.tensor_tensor(_sap(ch, 6, 3), _sap(ch, 6, 3), _sap(ch, 12, 3), OP.subtract)  # NB
    v.tensor_tensor(_sap(ch, 3, 3), _bap(ch, 0, 3), _sap(ch, 6, 3), OP.mult)       # d3
    v.tensor_tensor(_sap(ch, 9, 2), _bap(ch, 1, 2), _bap(ch, 5, 2, -2), OP.mult)
    v.tensor_tensor(_sap(ch, 12, 2), _bap(ch, 2, 2, -2), _bap(ch, 4, 2), OP.mult)
    v.tensor_tensor(_sap(ch, 11, 2, 3), _bap(ch, 0, 2), _bap(ch, 4, 2, -1), OP.mult)
    v.tensor_tensor(_sap(ch, 9, 3), _sap(ch, 9, 3), _sap(ch, 12, 3), OP.subtract)  # NA
    v.tensor_tensor(_sap(ch, 6, 3), _sap(ch, 9, 3), _sap(ch, 6, 3), OP.mult)       # x3
    # joint sums (plane, block, row) on both sides
    sum_dst = _ap(ch.tla, 0, [[ch.nb * G, 3], [G, ch.nb], [1, G]])
    v.tensor_tensor(
        sum_dst,
        _ap(ch.sca, 0, [[3 * G, 3], [SS, ch.nb], [1, G]]),
        _ap(ch.sca, G, [[3 * G, 3], [SS, ch.nb], [1, G]]),
        OP.add,
    )
    v.tensor_tensor(
        sum_dst, sum_dst, _ap(ch.sca, 2 * G, [[3 * G, 3], [SS, ch.nb], [1, G]]), OP.add
    )
    v.reciprocal_approx_fast(_tap(ch, 6), _tap(ch, 2))


def _emit_tail_a(nc, ch):
    """ACT burst (one sqrt-set visit, emitted after the next chunk's copies so
    the ACT queue never blocks on this chunk's DVE sums), then
    t = det*rd*sq; s2/t2; arctan.  sign(yy) == sign(det)."""
    v, s = nc.vector, nc.scalar
    s.activation(_tap(ch, 3), _tap(ch, 0), AF.Sqrt)
    s.activation(_tap(ch, 4), _tap(ch, 2), AF.Sign)
    s.activation(_tap(ch, 5), _tap(ch, 1), AF.Sign)
    v.tensor_tensor(_tap(ch, 7), _tap(ch, 1), _tap(ch, 6), OP.mult)
    v.tensor_tensor(_tap(ch, 7), _tap(ch, 7), _tap(ch, 3), OP.mult)
    v.tensor_tensor(_tap(ch, 9), _tap(ch, 4), _tap(ch, 5), OP.mult)
    v.tensor_tensor(_tap(ch, 9), _tap(ch, 5), _tap(ch, 9), OP.subtract)
    s.activation(_tap(ch, 8), _tap(ch, 7), AF.Arctan)


def _emit_tail_b(nc, ch, y):
    v = nc.vector
    v.scalar_tensor_tensor(
        _ap(ch.oba, 0, [[G, ch.nb], [1, G]]),
        _tap(ch, 9), PI / 2, _tap(ch, 8), OP.mult, OP.add,
    )
    nc.scalar.dma_start(
        out=y.rearrange("(p q) -> p q", p=128)[:, ch.b0 * G : (ch.b0 + ch.nb) * G],
        in_=_ap(ch.oba, 0, [[1, ch.nb * G]]),
    )


def build_kernel(atoms):
    c0, c1, c2, c3 = (3 * int(a) for a in atoms)
    nc = bacc.Bacc("TRN2", target_bir_lowering=False, debug=False)
    x = nc.dram_tensor("x", [ROWS_PER_CORE, 42], F32, kind="ExternalInput")
    y = nc.dram_tensor("y", [ROWS_PER_CORE], F32, kind="ExternalOutput")
    xr = x.rearrange("(p q) c -> p q c", p=128)
    with TileContext(nc) as tc:
        with (
            tc.tile_pool(name="xp", bufs=2) as xp,
            tc.tile_pool(name="cp", bufs=2) as cp,
            tc.tile_pool(name="bp", bufs=3) as bp,
            tc.tile_pool(name="sp", bufs=1) as sp,
            tc.tile_pool(name="tp", bufs=2) as tp,
            tc.tile_pool(name="op", bufs=2) as op,
        ):
            prev = None
            b = 0
            for nb in CHUNKS:
                bb = bp.tile([128, BS * 2], F32, tag="b")
                for j in range(nb):
                    xt = xp.tile([128, G * 42], F32, tag="x")
                    halves = b in (0, 1, T - 1)
                    if halves:
                        H = G // 2
                        nc.sync.dma_start(
                            out=_ap(xt[:], 0, [[1, H * 42]]),
                            in_=xr[:, b * G : b * G + H, :],
                        )
                        nc.sync.dma_start(
                            out=_ap(xt[:], H * 42, [[1, (G - H) * 42]]),
                            in_=xr[:, b * G + H : (b + 1) * G, :],
                        )
                    else:
                        nc.sync.dma_start(out=xt[:], in_=xr[:, b * G : (b + 1) * G, :])
                    ct = cp.tile([128, 12 * G], F32, tag="c")
                    _emit_ph1(nc, bb[:], ct[:], xt[:], j, c0, c1, c2, c3, halves)
                    b += 1
                sc = sp.tile([128, SS * 2], F32, tag="sc")
                tl = tp.tile([128, TP * G * 2], F32, tag="tl")
                ob = op.tile([128, G * 2], F32, tag="o")
                ch = Chunk(b - nb, nb, bb[:], sc[:], tl[:], ob[:])
                if prev is not None:
                    _emit_tail_a(nc, prev)
                _emit_head(nc, ch)
                if prev is not None:
                    _emit_tail_b(nc, prev, y)
                prev = ch
            _emit_tail_a(nc, prev)
            _emit_tail_b(nc, prev, y)
    nc.finalize()
    return nc


_CACHE = {}


def _get_nc(atoms):
    key = tuple(int(a) for a in atoms)
    if key not in _CACHE:
        _CACHE[key] = build_kernel(key)
    return _CACHE[key]


def run(x, atoms=(0, 4, 7, 11), **spmd_kwargs):
    """x: [B, 42] f32. Returns (y [B] f32, BassKernelResults)."""
    x = np.ascontiguousarray(np.asarray(x, dtype=np.float32))
    B = x.shape[0]
    total = N_CORES * ROWS_PER_CORE
    if B < total:
        # pad with replicated leading rows (valid, non-degenerate data)
        reps = -(-(total - B) // B)
        x = np.concatenate([x] + [x] * reps, axis=0)[:total]
    nc = _get_nc(atoms)
    shards = x.reshape(N_CORES, ROWS_PER_CORE, 42)
    in_maps = [{"x": shards[i]} for i in range(N_CORES)]
    res = run_bass_kernel_spmd(nc, in_maps, core_ids=list(range(N_CORES)), **spmd_kwargs)
    y = np.concatenate([r["y"] for r in res.results])[:B]
    return np.asarray(y, dtype=np.float32), res


def kernel(x, mask_matrix):
    mask = np.asarray(mask_matrix)
    atoms = tuple(int(i) for i in np.argmax(mask, axis=1))
    y, _ = run(x, atoms=atoms)
    return y


# revision 13
# speedup vs baseline: 1.1092x; 1.0238x over previous
"""Trainium2 Bass kernel for nn_DihedralAngleLayer (planar v5).

Input:  x [2_000_000, 42] f32 (14 atoms x 3 coords per row),
        mask_matrix [4, 14] f32 one-hot carbon selector.
Output: dihedral angle per row, [2_000_000] f32.

Data-parallel across 8 NeuronCores; rows padded to 8*128*1960 and split.
Per core, rows are partition-major: partition p owns rows [p*Q, (p+1)*Q).

Per uniform block (G=196 rows/partition) the row-major tile is planarized by
TWO transposing copies on the Scalar/ACT engine (carbon pairs share a
uniform column stride, so each copy moves 2 carbons x 3 components in one
3-dim-AP op), then ONE contiguous DVE subtract forms all nine bond-vector
planes: (u,v,w) = carbons[1:4] - carbons[0:3].  This keeps strided access
patterns off the Vector engine (f32 TT has no DVE fast mode; strided
single-element runs cost ~2x, measured).

Phase 2 per chunk of blocks (planar unit-stride, 16 DVE instrs):
    NB = v x w,  NA = u x v            (pairwise plane mults, P1-P2)
    q3 = v*v (ACT Square), d3 = u*NB, x3 = NA*NB (in-place over NB)
    (q, det, xx) = joint 3-group pair-sum                (2 strided adds)
    one ACT burst: sq = sqrt(q), sgx = sign(xx), sgy = sign(det)
    tail (direct atan2 -- the ACT arctan table saturates correctly for
    huge arguments):  t = det*recip(xx)*sq;  A = arctan(t)
        phi = A + (pi/2)*(sgy - sgx*sgy)

The emission is software-pipelined: chunk c's post-arctan tail is emitted
after chunk c+1's phase-1 and head, so the DVE never stalls on the ACT
round-trips (sqrt/arctan sit behind ~1.3us table swaps: no ACT table set
holds both sqrt and arctan).  Input loads alternate between the two HWDGE
rings (nc.sync / nc.scalar) so two DMAs stream concurrently; GPSIMD stays
fully idle (concurrent GPSIMD tensor/SWDGE work measurably degrades DVE
and the DMA stream).
"""

import numpy as np

import concourse.bacc as bacc
import concourse.bass as bass
import concourse.mybir as mybir
from concourse.bass_utils import run_bass_kernel_spmd
from concourse.tile import TileContext

AF = mybir.ActivationFunctionType
OP = mybir.AluOpType
F32 = mybir.dt.float32

PI = float(np.pi)

N_CORES = 8
G = 196
T = 10
Q = G * T                   # 1960 rows per partition
ROWS_PER_CORE = 128 * Q     # 250880
CHUNKS = [1, 2, 2, 2, 2, 1]

BS = 9 * G                  # b-plane block stride within a chunk tile
SP = 15                     # head scratch planes per block
SS = SP * G                 # head scratch block stride
TP = 10                     # tail planes per chunk


def _ap(base, off, dims):
    return bass.AP(
        base.tensor, base.offset + off, [list(base.ap[0])] + [list(d) for d in dims]
    )


def _emit_ph1(nc, bba, cpa, xta, j, c0, c1, c2, c3, halves=False):
    """ACT transposing copies -> 12 carbon planes; DVE mega-sub -> 9 b-planes
    at local block index j of the chunk's b-tile.  With halves=True the
    copies are row-split so they start at half-block DMA arrival."""
    s, v = nc.scalar, nc.vector
    parts = ((0, G // 2), (G // 2, G - G // 2)) if halves else ((0, G),)
    for r0, rn in parts:
        s.activation(
            _ap(cpa, r0, [[3 * G, 2], [G, 3], [1, rn]]),
            _ap(xta, c0 + r0 * 42, [[c1 - c0, 2], [1, 3], [42, rn]]),
            AF.Copy,
        )
        s.activation(
            _ap(cpa, 6 * G + r0, [[3 * G, 2], [G, 3], [1, rn]]),
            _ap(xta, c2 + r0 * 42, [[c3 - c2, 2], [1, 3], [42, rn]]),
            AF.Copy,
        )
    v.tensor_tensor(
        _ap(bba, j * BS, [[1, 9 * G]]),
        _ap(cpa, 3 * G, [[1, 9 * G]]),
        _ap(cpa, 0, [[1, 9 * G]]),
        OP.subtract,
    )


class Chunk:
    def __init__(self, b0, nb, bba, sca, tla, oba):
        self.b0, self.nb = b0, nb
        self.bba, self.sca, self.tla, self.oba = bba, sca, tla, oba


def _bap(ch, plane, n=1, ps=1):
    dims = [[BS, ch.nb]] + ([[ps * G, n]] if n > 1 else []) + [[1, G]]
    return _ap(ch.bba, plane * G, dims)


def _sap(ch, plane, n=1, ps=1):
    dims = [[SS, ch.nb]] + ([[ps * G, n]] if n > 1 else []) + [[1, G]]
    return _ap(ch.sca, plane * G, dims)


def _tap(ch, plane):
    return _ap(ch.tla, plane * ch.nb * G, [[G, ch.nb], [1, G]])


def _emit_head(nc, ch):
    """Head: everything up to (and including) recip(xx).
    Scratch: q3=A(0-2), d3=D(3-5), NB=C(6-8), NA=B(9-11), P2=P(12-14).
    Tail tile: q(0) det(1) xx(2) sq(3) sgx(4) sgy(5) rd(6) t(7) A(8) s2(9)."""
    v, s = nc.vector, nc.scalar
    s.activation(_sap(ch, 0, 3), _bap(ch, 3, 3), AF.Square)                 # q3
    v.tensor_tensor(_sap(ch, 6, 2), _bap(ch, 4, 2), _bap(ch, 8, 2, -2), OP.mult)
    v.tensor_tensor(_sap(ch, 12, 2), _bap(ch, 5, 2, -2), _bap(ch, 7, 2), OP.mult)
    v.tensor_tensor(_sap(ch, 8, 2, 6), _bap(ch, 3, 2), _bap(ch, 7, 2, -1), OP.mult)
    v.tensor_tensor(_sap(ch, 6, 3), _sap(ch, 6, 3), _sap(ch, 12, 3), OP.subtract)  # NB
    v.tensor_tensor(_sap(ch, 3, 3), _bap(ch, 0, 3), _sap(ch, 6, 3), OP.mult)       # d3
    v.tensor_tensor(_sap(ch, 9, 2), _bap(ch, 1, 2), _bap(ch, 5, 2, -2), OP.mult)
    v.tensor_tensor(_sap(ch, 12, 2), _bap(ch, 2, 2, -2), _bap(ch, 4, 2), OP.mult)
    v.tensor_tensor(_sap(ch, 11, 2, 3), _bap(ch, 0, 2), _bap(ch, 4, 2, -1), OP.mult)
    v.tensor_tensor(_sap(ch, 9, 3), _sap(ch, 9, 3), _sap(ch, 12, 3), OP.subtract)  # NA
    v.tensor_tensor(_sap(ch, 6, 3), _sap(ch, 9, 3), _sap(ch, 6, 3), OP.mult)       # x3
    # joint sums (plane, block, row) on both sides
    sum_dst = _ap(ch.tla, 0, [[ch.nb * G, 3], [G, ch.nb], [1, G]])
    v.tensor_tensor(
        sum_dst,
        _ap(ch.sca, 0, [[3 * G, 3], [SS, ch.nb], [1, G]]),
        _ap(ch.sca, G, [[3 * G, 3], [SS, ch.nb], [1, G]]),
        OP.add,
    )
    v.tensor_tensor(
        sum_dst, sum_dst, _ap(ch.sca, 2 * G, [[3 * G, 3], [SS, ch.nb], [1, G]]), OP.add
    )
    v.reciprocal_approx_fast(_tap(ch, 6), _tap(ch, 2))


def _emit_tail_a(nc, ch):
    """ACT burst (one sqrt-set visit, emitted after the next chunk's copies so
    the ACT queue never blocks on this chunk's DVE sums), then
    t = det*rd*sq; s2/t2; arctan.  sign(yy) == sign(det)."""
    v, s = nc.vector, nc.scalar
    s.activation(_tap(ch, 3), _tap(ch, 0), AF.Sqrt)
    s.activation(_tap(ch, 4), _tap(ch, 2), AF.Sign)
    s.activation(_tap(ch, 5), _tap(ch, 1), AF.Sign)
    v.tensor_tensor(_tap(ch, 7), _tap(ch, 1), _tap(ch, 6), OP.mult)
    v.tensor_tensor(_tap(ch, 7), _tap(ch, 7), _tap(ch, 3), OP.mult)
    v.tensor_tensor(_tap(ch, 9), _tap(ch, 4), _tap(ch, 5), OP.mult)
    v.tensor_tensor(_tap(ch, 9), _tap(ch, 5), _tap(ch, 9), OP.subtract)
    s.activation(_tap(ch, 8), _tap(ch, 7), AF.Arctan)


def _emit_tail_b(nc, ch, y):
    v = nc.vector
    v.scalar_tensor_tensor(
        _ap(ch.oba, 0, [[G, ch.nb], [1, G]]),
        _tap(ch, 9), PI / 2, _tap(ch, 8), OP.mult, OP.add,
    )
    nc.scalar.dma_start(
        out=y.rearrange("(p q) -> p q", p=128)[:, ch.b0 * G : (ch.b0 + ch.nb) * G],
        in_=_ap(ch.oba, 0, [[1, ch.nb * G]]),
    )


def build_kernel(atoms):
    c0, c1, c2, c3 = (3 * int(a) for a in atoms)
    nc = bacc.Bacc("TRN2", target_bir_lowering=False, debug=False)
    x = nc.dram_tensor("x", [ROWS_PER_CORE, 42], F32, kind="ExternalInput")
    y = nc.dram_tensor("y", [ROWS_PER_CORE], F32, kind="ExternalOutput")
    xr = x.rearrange("(p q) c -> p q c", p=128)
    with TileContext(nc) as tc:
        with (
            tc.tile_pool(name="xp", bufs=2) as xp,
            tc.tile_pool(name="cp", bufs=2) as cp,
            tc.tile_pool(name="bp", bufs=3) as bp,
            tc.tile_pool(name="sp", bufs=1) as sp,
            tc.tile_pool(name="tp", bufs=2) as tp,
            tc.tile_pool(name="op", bufs=2) as op,
        ):
            prev = None
            b = 0
            for nb in CHUNKS:
                bb = bp.tile([128, BS * 2], F32, tag="b")
                for j in range(nb):
                    xt = xp.tile([128, G * 42], F32, tag="x")
                    halves = True
                    if halves:
                        H = G // 2
                        nc.sync.dma_start(
                            out=_ap(xt[:], 0, [[1, H * 42]]),
                            in_=xr[:, b * G : b * G + H, :],
                        )
                        nc.sync.dma_start(
                            out=_ap(xt[:], H * 42, [[1, (G - H) * 42]]),
                            in_=xr[:, b * G + H : (b + 1) * G, :],
                        )
                    else:
                        nc.sync.dma_start(out=xt[:], in_=xr[:, b * G : (b + 1) * G, :])
                    ct = cp.tile([128, 12 * G], F32, tag="c")
                    _emit_ph1(nc, bb[:], ct[:], xt[:], j, c0, c1, c2, c3, halves)
                    b += 1
                sc = sp.tile([128, SS * 2], F32, tag="sc")
                tl = tp.tile([128, TP * G * 2], F32, tag="tl")
                ob = op.tile([128, G * 2], F32, tag="o")
                ch = Chunk(b - nb, nb, bb[:], sc[:], tl[:], ob[:])
                if prev is not None:
                    _emit_tail_a(nc, prev)
                _emit_head(nc, ch)
                if prev is not None:
                    _emit_tail_b(nc, prev, y)
                prev = ch
            _emit_tail_a(nc, prev)
            _emit_tail_b(nc, prev, y)
    nc.finalize()
    return nc


_CACHE = {}


def _get_nc(atoms):
    key = tuple(int(a) for a in atoms)
    if key not in _CACHE:
        _CACHE[key] = build_kernel(key)
    return _CACHE[key]


def run(x, atoms=(0, 4, 7, 11), **spmd_kwargs):
    """x: [B, 42] f32. Returns (y [B] f32, BassKernelResults)."""
    x = np.ascontiguousarray(np.asarray(x, dtype=np.float32))
    B = x.shape[0]
    total = N_CORES * ROWS_PER_CORE
    if B < total:
        # pad with replicated leading rows (valid, non-degenerate data)
        reps = -(-(total - B) // B)
        x = np.concatenate([x] + [x] * reps, axis=0)[:total]
    nc = _get_nc(atoms)
    shards = x.reshape(N_CORES, ROWS_PER_CORE, 42)
    in_maps = [{"x": shards[i]} for i in range(N_CORES)]
    res = run_bass_kernel_spmd(nc, in_maps, core_ids=list(range(N_CORES)), **spmd_kwargs)
    y = np.concatenate([r["y"] for r in res.results])[:B]
    return np.asarray(y, dtype=np.float32), res


def kernel(x, mask_matrix):
    mask = np.asarray(mask_matrix)
    atoms = tuple(int(i) for i in np.argmax(mask, axis=1))
    y, _ = run(x, atoms=atoms)
    return y
